# revision 32
# baseline (speedup 1.0000x reference)
"""Trainium2 Bass kernel v2 for nn_Conv1dMultiscaleLocalization.

Per image [768,768], one image per core (B=8 data-parallel):
  resp_j = vconv(C, k_j) + hconv(S, k_j);  conv = max_j resp_j
  pooled = 11x11 max pool; mask = (conv==mw(conv)) & (conv>0.5)

Same numerics as v1 (bf16 hi+lo exact split, fp32 PSUM; 0 mask flips).
Perf changes vs v1 (trace-driven):
  - ldweights=False on consecutive same-stationary matmuls: H conv was
    LDWEIGHTS-gated (600 small matmuls each paying a ~116ns weight load);
    now ~1 load per stationary chunk.  Matmul order restructured (terms and
    halves grouped per stationary) to maximize reuse runs.
  - Batched DMA: c96 stays block-major; st becomes wave-major so each wave's
    H stationaries arrive in one [128,768] transfer; ~30 DMAs vs ~100.
  - mwv (vertical pool window) batched across all 6 column chunks per step
    via one 3D-AP instruction instead of 6, on a single atg tile.
"""
import sys
import numpy as np

sys.path.insert(0, "/opt/trn_rl_repo")

import ml_dtypes  # noqa: E402
import concourse.bacc as bacc  # noqa: E402
import concourse.mybir as mybir  # noqa: E402
import concourse.tile as tile  # noqa: E402
from concourse.bass_utils import run_bass_kernel_spmd  # noqa: E402

F32 = mybir.dt.float32
BF16 = mybir.dt.bfloat16
U8 = mybir.dt.uint8
AF = mybir.ActivationFunctionType
ALU = mybir.AluOpType

H = W = 768
KERNEL_SIZES = [3, 9, 15, 21, 31, 51, 65]
NJ = 7
XJ = [(w - 1) // 2 for w in KERNEL_SIZES]
SCALES = [1.0 / (w - 1) for w in KERNEL_SIZES]
NB = 6          # 128-row blocks per image
NEG = -3.0e38
NTERMS = 2      # bf16 split terms (hi, lo)
# one j per PSUM wave: the combine/evac chain starts as soon as j0's
# matmuls stop instead of after the whole 3-j group (kills the ~12us
# DVE startup bubble seen in the trace); same matmul/op counts.
JG = [(j, j + 1) for j in range(NJ)]
C0 = float(np.nextafter(np.float32(0.5), np.float32(1.0)))  # >0.5 threshold

_CACHE = {}


# ---------------------------------------------------------------- constants
def _sign_band(d, x):
    return np.where((d >= -x) & (d <= -1), 1.0,
                    np.where((d >= 1) & (d <= x), -1.0, 0.0))


def _toeplitz_v2():
    """V stationary [128, NJ*3, 128]: K packs (64 v-rows x 2 terms) by
    partition parity; chunk k covers input rows 128b-32+64k + p//2.
    T2[p, 3j+k, m] = band_j((-32 + 64k + p//2) - m)."""
    T = np.zeros((128, NJ * 3, 128), dtype=np.float32)
    p = np.arange(128)[:, None]
    m = np.arange(128)[None, :]
    for j in range(NJ):
        for k in range(3):
            T[:, 3 * j + k, :] = _sign_band((-32 + 64 * k + p // 2) - m, XJ[j])
    return T


def _band_h2():
    """H moving [128, NJ*128]: K packs (64 w'-cols x 2 terms); chunk a covers
    w' = 64a + p//2, out col w = 64a - 32 + n.
    T2[p, 128j+n] = band_j(p//2 + 32 - n)."""
    T = np.zeros((128, NJ * 128), dtype=np.float32)
    p = np.arange(128)[:, None]
    n = np.arange(128)[None, :]
    for j in range(NJ):
        T[:, 128 * j:128 * (j + 1)] = _sign_band(p // 2 + 32 - n, XJ[j])
    return T


def _split_terms(x):
    terms = []
    r = x
    for _ in range(NTERMS):
        t = r.astype(ml_dtypes.bfloat16)
        terms.append(t)
        r = r - t.astype(np.float32)
    return terms


def _interleave(t0, t1):
    """[R, ...] x2 -> [2R, ...] with rows (2r, 2r+1) = (t0[r], t1[r])."""
    out = np.empty((t0.shape[0] * 2,) + t0.shape[1:], dtype=t0.dtype)
    out[0::2] = t0
    out[1::2] = t1
    return out


def _prep_core(Cb, Sb):
    """c2 [128, NB*3, W]: c2[p, 3b+k, n] = term_{p%2}(C)[128b-32+64k+p//2, n];
    stw2 [128, NB, 12, 128]: stw2[p, ib, a, m] = term_{p%2}(S)[128ib+m,
    64a+p//2]."""
    ct = _split_terms(Cb)
    cint = _interleave(ct[0].astype(np.float32),
                       ct[1].astype(np.float32))  # [1536, W] rows 2v+t
    cpad = np.vstack([np.zeros((64, W), np.float32), cint,
                      np.zeros((192, W), np.float32)])  # row 2(v+32)+t
    c2 = np.zeros((128, NB * 3, W), dtype=np.float32)
    for b in range(NB):
        for k in range(3):
            r0 = 2 * (128 * b + 64 * k)  # = 2*(v0+32) with v0 = 128b-32+64k
            c2[:, 3 * b + k, :] = cpad[r0:r0 + 128, :]
    st = _split_terms(Sb.T)  # [w', u]
    sint = _interleave(st[0].astype(np.float32),
                       st[1].astype(np.float32))  # [1536, u] rows 2w'+t
    stw2 = sint.reshape(12, 128, NB, 128).transpose(1, 2, 0, 3)
    return {"c2": c2.astype(ml_dtypes.bfloat16).reshape(128, -1),
            "stw2": np.ascontiguousarray(stw2).astype(
                ml_dtypes.bfloat16).reshape(128, -1)}


def _consts():
    return {
        "TV2": _toeplitz_v2().astype(ml_dtypes.bfloat16).reshape(128, -1),
        "TH2": _band_h2().astype(ml_dtypes.bfloat16).reshape(128, -1),
        "IDT": np.eye(128, dtype=np.float32),
    }


# ---------------------------------------------------------------- kernel IR
def _build():
    nc = bacc.Bacc()
    C2D = nc.declare_dram_parameter("c2", [128, NB * 3 * W], BF16,
                                    isOutput=False)
    STW2 = nc.declare_dram_parameter("stw2", [128, NB * 12 * 128], BF16,
                                     isOutput=False)
    TV2D = nc.declare_dram_parameter("TV2", [128, NJ * 3 * 128], BF16,
                                     isOutput=False)
    TH2D = nc.declare_dram_parameter("TH2", [128, NJ * 128], BF16,
                                     isOutput=False)
    IDT = nc.declare_dram_parameter("IDT", [128, 128], F32, isOutput=False)
    CONV = nc.declare_dram_parameter("conv", [H, W], F32, isOutput=True)
    MASK = nc.declare_dram_parameter("mask", [H, W], U8, isOutput=True)

    def hspan(a, j):
        # out-col span covered by w'-chunk a (64 cols, both terms)
        return max(0, 64 * a - XJ[j]), min(W, 64 * a + 64 + XJ[j])

    with tile.TileContext(nc) as tc:
        with tc.tile_pool(name="big", bufs=1) as big, \
             tc.tile_pool(name="consts", bufs=1) as cst, \
             tc.tile_pool(name="posg", bufs=1) as posp, \
             tc.tile_pool(name="pool", bufs=2) as poolp, \
             tc.tile_pool(name="atg", bufs=1) as atgp, \
             tc.tile_pool(name="pooled", bufs=1) as pooledp, \
             tc.tile_pool(name="small", bufs=2) as smallp, \
             tc.tile_pool(name="ps", bufs=3, space="PSUM") as ps, \
             tc.tile_pool(name="psT", bufs=2, space="PSUM") as psT:

            c2 = big.tile([128, NB * 3, W], BF16, tag="c2", name="c2")
            stw2 = big.tile([128, NB, 12, 128], BF16, tag="stw2", name="stw2")
            tv2 = cst.tile([128, NJ * 3, 128], BF16, tag="tv2")
            th2 = cst.tile([128, NJ * 128], BF16, tag="th2")
            idt = cst.tile([128, 128], F32, tag="idt")
            # block-priority loads on BOTH hwdge queues: wave ib needs
            # c2[:, 3ib:3ib+3, :] (V) and stw2[:, ib, :, :] (H stationaries).
            # First-wave critical path: TV2 j0-2 slice, then c2 b0 per k-chunk.
            nc.sync.dma_start(out=tv2[:, 0:3, :], in_=TV2D[:, 0:3 * 128])
            nc.scalar.dma_start(out=th2[:], in_=TH2D[:])
            for k in range(3):
                nc.sync.dma_start(out=c2[:, k:k + 1, :],
                                  in_=C2D[:, k * W:(k + 1) * W])
            nc.sync.dma_start(out=tv2[:, 3:21, :], in_=TV2D[:, 3 * 128:])
            nc.scalar.dma_start(out=stw2[:, 0, :, :], in_=STW2[:, 0:12 * 128])
            nc.scalar.dma_start(out=idt[:], in_=IDT[:])
            for b in range(1, NB):
                nc.sync.dma_start(out=c2[:, 3 * b:3 * (b + 1), :],
                                  in_=C2D[:, 3 * b * W:3 * (b + 1) * W])
                nc.scalar.dma_start(
                    out=stw2[:, b, :, :],
                    in_=STW2[:, 12 * 128 * b:12 * 128 * (b + 1)])

            posg = [posp.tile([128, 800], F32, tag=f"posg{ib}", name=f"posg{ib}")
                    for ib in range(NB)]
            # pad memsets hoisted into the startup bubble (combine only
            # writes [16:784], so the NEG pads persist per block)
            for ib in range(NB):
                nc.vector.memset(posg[ib][:, 0:16], NEG)
                nc.vector.memset(posg[ib][:, 784:800], NEG)
            atg = atgp.tile([128, NB, 800], F32, tag="atg", name="atg")
            ptv = pooledp.tile([128, NB, W], F32, tag="ptv", name="ptv")
            nc.vector.memset(atg[:, :, 0:16], NEG)
            nc.vector.memset(atg[:, :, 784:800], NEG)

            def last_a(j, h):
                lo_h, hi_h = 384 * h, 384 * (h + 1)
                return max(a for a in range(12)
                           if max(hspan(a, j)[0], lo_h)
                           < min(hspan(a, j)[1], hi_h))

            def emit_wave(ib):
                for (j0, j1) in JG:
                    # 2-bank tiles: [128, 2, 384] padded to [128, 2, 512] so
                    # each half sits bank-aligned; combine reads both at once.
                    ptiles = {j: ps.tile([128, 2, 384], F32, tag="p",
                                         name=f"p{j}",
                                         padded_shape=[128, 2, 512])
                              for j in range(j0, j1)}
                    # ---- V: 3 K-chunks (64 v-rows x 2 terms each) per half
                    for j in range(j0, j1):
                        for k in range(3):
                            for h in range(2):
                                rhs = c2[:, 3 * ib + k, 384 * h:384 * (h + 1)]
                                nc.tensor.matmul(
                                    ptiles[j][:, h, :], tv2[:, 3 * j + k, :],
                                    rhs, start=(k == 0), stop=False,
                                    skip_group_check=True)
                    # ---- H: stationary = stw2 chunk (ib, a); both terms ride
                    spans = {}
                    for a in range(12):
                        for j in range(j0, j1):
                            lo, hi = hspan(a, j)
                            if lo < hi:
                                spans[(a, j)] = (lo, hi)
                    lasts = {}
                    for (a, j) in spans:
                        lasts[j] = a
                    for a in range(12):
                        if not any((a, j) in spans for j in range(j0, j1)):
                            continue
                        lhs = stw2[:, ib, a, :]
                        first = True
                        for j in range(j0, j1):
                            if (a, j) not in spans:
                                continue
                            lo, hi = spans[(a, j)]
                            for h in range(2):
                                l2 = max(lo, 384 * h)
                                h2 = min(hi, 384 * (h + 1))
                                if l2 >= h2:
                                    continue
                                off = 128 * j + (l2 - (64 * a - 32))
                                stop = (lasts[j] == a and h2 == hi)
                                mm = nc.tensor.matmul(
                                    ptiles[j][:, h, l2 - 384 * h:h2 - 384 * h],
                                    lhs, th2[:, off:off + (h2 - l2)],
                                    start=False, stop=stop,
                                    skip_group_check=True)
                                if not first:
                                    mm.ins.ldweights = False
                                first = False
                    # ---- combine into posg, 768 wide via the 2-bank AP
                    dst = posg[ib][:, 16:784]
                    for j in range(j0, j1):
                        if j == 0:
                            nc.scalar.activation(dst, ptiles[0][:, :, :],
                                                 AF.Copy,
                                                 scale=float(SCALES[0]))
                        else:
                            nc.vector.scalar_tensor_tensor(
                                dst, ptiles[j][:, :, :], float(SCALES[j]),
                                dst, ALU.mult, ALU.max)

            def emit_mwh(ib):
                m2 = poolp.tile([128, 800], F32, tag="m2", name="m2")
                m4 = poolp.tile([128, 800], F32, tag="m4", name="m4")
                m8 = poolp.tile([128, 800], F32, tag="m8", name="m8")
                a = poolp.tile([128, W], F32, tag="a", name="a")
                g = posg[ib]
                nc.vector.tensor_tensor(m2[:, 11:788], g[:, 11:788],
                                        g[:, 12:789], ALU.max)
                nc.vector.tensor_tensor(m4[:, 11:786], m2[:, 11:786],
                                        m2[:, 13:788], ALU.max)
                nc.vector.tensor_tensor(m8[:, 11:779], m4[:, 11:779],
                                        m4[:, 15:783], ALU.max)
                nc.vector.tensor_tensor(a[:], m8[:, 11:779], m4[:, 18:786],
                                        ALU.max)
                return a

            def emit_at_half(ib, a, half):
                pt = psT.tile([128, 384], F32, tag="pt", name="pt")
                for k in range(3):
                    c = 3 * half + k
                    nc.tensor.transpose(pt[:, 128 * k:128 * (k + 1)],
                                        a[:, 128 * c:128 * (c + 1)], idt[:])
                nc.scalar.activation(
                    atg[:, 3 * half:3 * half + 3,
                        16 + 128 * ib:16 + 128 * (ib + 1)],
                    pt[:], AF.Copy)

            def emit_at(ib, a):
                for half in range(2):
                    emit_at_half(ib, a, half)

            def emit_mwh_half(g, m2, m4, m8, a, half):
                # same 11-window chain as emit_mwh, restricted to one
                # 384-col output half so at() transposes can interleave
                r0 = 11 + 384 * half
                nc.vector.tensor_tensor(m2[:, r0:r0 + 393], g[:, r0:r0 + 393],
                                        g[:, r0 + 1:r0 + 394], ALU.max)
                nc.vector.tensor_tensor(m4[:, r0:r0 + 391], m2[:, r0:r0 + 391],
                                        m2[:, r0 + 2:r0 + 393], ALU.max)
                nc.vector.tensor_tensor(m8[:, r0:r0 + 384], m4[:, r0:r0 + 384],
                                        m4[:, r0 + 4:r0 + 388], ALU.max)
                nc.vector.tensor_tensor(a[:, 384 * half:384 * half + 384],
                                        m8[:, r0:r0 + 384],
                                        m4[:, r0 + 7:r0 + 391], ALU.max)

            def emit_mwv(vib, c0=0, c1=NB):
                # vertical window max; [c0, c1) selects the column chunks so
                # the drain tail can interleave with ptt transposes per half
                av = 16 + 128 * vib
                n = 128
                m2 = poolp.tile([128, NB, 272], F32, tag="m2v", name="m2v")
                m4 = poolp.tile([128, NB, 272], F32, tag="m4v", name="m4v")
                m8 = poolp.tile([128, NB, 272], F32, tag="m8v", name="m8v")
                nc.vector.tensor_tensor(m2[:, c0:c1, 0:n + 16],
                                        atg[:, c0:c1, av - 8:av + n + 8],
                                        atg[:, c0:c1, av - 7:av + n + 9],
                                        ALU.max)
                nc.vector.tensor_tensor(m4[:, c0:c1, 0:n + 14],
                                        m2[:, c0:c1, 0:n + 14],
                                        m2[:, c0:c1, 2:n + 16], ALU.max)
                nc.vector.tensor_tensor(m8[:, c0:c1, 2:n + 10],
                                        m4[:, c0:c1, 2:n + 10],
                                        m4[:, c0:c1, 6:n + 14], ALU.max)
                nc.vector.tensor_tensor(ptv[:, c0:c1, 128 * vib:128 * vib + n],
                                        m8[:, c0:c1, 3:n + 3],
                                        m4[:, c0:c1, 10:n + 10], ALU.max)

            def emit_ptt_half(vib, half, mk):
                # fused mask per half, reading the transposed pooled tile
                # straight from PSUM (one PSUM operand is allowed):
                #   mask = (pooled max nextafter(0.5)) <= posg
                # pooled >= posg always, so <= means equality (local max)
                # AND posg > 0.5 via the raised scalar.
                pt = psT.tile([128, 384], F32, tag="pt", name="pt")
                for k in range(3):
                    c = 3 * half + k
                    nc.tensor.transpose(
                        pt[:, 128 * k:128 * (k + 1)],
                        ptv[:, c, 128 * vib:128 * (vib + 1)], idt[:])
                nc.vector.scalar_tensor_tensor(
                    mk[:, 384 * half:384 * (half + 1)], pt[:], C0,
                    posg[vib][:, 16 + 384 * half:16 + 384 * (half + 1)],
                    ALU.max, ALU.is_le)

            def emit_ptt(vib):
                mk = smallp.tile([128, W], U8, tag="mk", name="mk")
                emit_ptt_half(vib, 0, mk)
                emit_ptt_half(vib, 1, mk)
                nc.scalar.dma_start(out=MASK[128 * vib:128 * (vib + 1), :],
                                  in_=mk[:])

            alist = {}
            for ib in range(NB):
                emit_wave(ib)
                if ib >= 1:
                    emit_at(ib - 1, alist[ib - 1])
                if ib >= 2:
                    emit_mwv(ib - 2)
                    emit_ptt(ib - 2)
                nc.sync.dma_start(out=CONV[128 * ib:128 * (ib + 1), :],
                                  in_=posg[ib][:, 16:784])
                if ib < NB - 1:
                    alist[ib] = emit_mwh(ib)
                else:
                    # drain tail: mwh halves interleaved with at halves so
                    # the PE transposes overlap the DVE window-max chain
                    m2 = poolp.tile([128, 800], F32, tag="m2", name="m2")
                    m4 = poolp.tile([128, 800], F32, tag="m4", name="m4")
                    m8 = poolp.tile([128, 800], F32, tag="m8", name="m8")
                    a = poolp.tile([128, W], F32, tag="a", name="a")
                    g = posg[ib]
                    emit_mwh_half(g, m2, m4, m8, a, 0)
                    emit_at_half(ib, a, 0)
                    emit_mwh_half(g, m2, m4, m8, a, 1)
                    emit_at_half(ib, a, 1)
            # drain tail: split mwv by chunk-halves and interleave the ptt
            # transposes+mask so PE/DVE overlap instead of serializing
            for vib in (NB - 2, NB - 1):
                mk = smallp.tile([128, W], U8, tag="mk", name="mk")
                emit_mwv(vib, 0, 3)
                emit_ptt_half(vib, 0, mk)
                emit_mwv(vib, 3, 6)
                emit_ptt_half(vib, 1, mk)
                nc.scalar.dma_start(out=MASK[128 * vib:128 * (vib + 1), :],
                                    in_=mk[:])

    nc.compile()
    return nc


# ---------------------------------------------------------------- host glue
def kernel(C, S, kernel_cos, kernel_sin):
    C = np.asarray(C, dtype=np.float32)
    S = np.asarray(S, dtype=np.float32)
    B = C.shape[0]
    if "nc" not in _CACHE:
        _CACHE["nc"] = _build()
    nc = _CACHE["nc"]
    consts = _consts()
    in_maps = []
    for b in range(B):
        m = _prep_core(C[b, 0], S[b, 0])
        m.update(consts)
        in_maps.append(m)
    res = run_bass_kernel_spmd(nc, in_maps, core_ids=list(range(B)))
    conv = np.stack([r["conv"] for r in res.results])[:, None]
    mask = np.stack([r["mask"] for r in res.results])[:, None].astype(bool)
    return conv.astype(np.float32), mask



# revision 34
# speedup vs baseline: 1.0036x; 1.0036x over previous
"""Trainium2 Bass kernel v2 for nn_Conv1dMultiscaleLocalization.

Per image [768,768], one image per core (B=8 data-parallel):
  resp_j = vconv(C, k_j) + hconv(S, k_j);  conv = max_j resp_j
  pooled = 11x11 max pool; mask = (conv==mw(conv)) & (conv>0.5)

Same numerics as v1 (bf16 hi+lo exact split, fp32 PSUM; 0 mask flips).
Perf changes vs v1 (trace-driven):
  - ldweights=False on consecutive same-stationary matmuls: H conv was
    LDWEIGHTS-gated (600 small matmuls each paying a ~116ns weight load);
    now ~1 load per stationary chunk.  Matmul order restructured (terms and
    halves grouped per stationary) to maximize reuse runs.
  - Batched DMA: c96 stays block-major; st becomes wave-major so each wave's
    H stationaries arrive in one [128,768] transfer; ~30 DMAs vs ~100.
  - mwv (vertical pool window) batched across all 6 column chunks per step
    via one 3D-AP instruction instead of 6, on a single atg tile.
"""
import sys
import numpy as np

sys.path.insert(0, "/opt/trn_rl_repo")

import ml_dtypes  # noqa: E402
import concourse.bacc as bacc  # noqa: E402
import concourse.mybir as mybir  # noqa: E402
import concourse.tile as tile  # noqa: E402
from concourse.bass_utils import run_bass_kernel_spmd  # noqa: E402

F32 = mybir.dt.float32
BF16 = mybir.dt.bfloat16
U8 = mybir.dt.uint8
AF = mybir.ActivationFunctionType
ALU = mybir.AluOpType

H = W = 768
KERNEL_SIZES = [3, 9, 15, 21, 31, 51, 65]
NJ = 7
XJ = [(w - 1) // 2 for w in KERNEL_SIZES]
SCALES = [1.0 / (w - 1) for w in KERNEL_SIZES]
NB = 6          # 128-row blocks per image
NEG = -3.0e38
NTERMS = 2      # bf16 split terms (hi, lo)
# one j per PSUM wave: the combine/evac chain starts as soon as j0's
# matmuls stop instead of after the whole 3-j group (kills the ~12us
# DVE startup bubble seen in the trace); same matmul/op counts.
JG = [(j, j + 1) for j in range(NJ)]
C0 = float(np.nextafter(np.float32(0.5), np.float32(1.0)))  # >0.5 threshold

_CACHE = {}


# ---------------------------------------------------------------- constants
def _sign_band(d, x):
    return np.where((d >= -x) & (d <= -1), 1.0,
                    np.where((d >= 1) & (d <= x), -1.0, 0.0))


def _toeplitz_v2():
    """V stationary [128, NJ*3, 128]: K packs (64 v-rows x 2 terms) by
    partition parity; chunk k covers input rows 128b-32+64k + p//2.
    T2[p, 3j+k, m] = band_j((-32 + 64k + p//2) - m)."""
    T = np.zeros((128, NJ * 3, 128), dtype=np.float32)
    p = np.arange(128)[:, None]
    m = np.arange(128)[None, :]
    for j in range(NJ):
        for k in range(3):
            T[:, 3 * j + k, :] = _sign_band((-32 + 64 * k + p // 2) - m, XJ[j])
    return T


def _band_h2():
    """H moving [128, NJ*128]: K packs (64 w'-cols x 2 terms); chunk a covers
    w' = 64a + p//2, out col w = 64a - 32 + n.
    T2[p, 128j+n] = band_j(p//2 + 32 - n)."""
    T = np.zeros((128, NJ * 128), dtype=np.float32)
    p = np.arange(128)[:, None]
    n = np.arange(128)[None, :]
    for j in range(NJ):
        T[:, 128 * j:128 * (j + 1)] = _sign_band(p // 2 + 32 - n, XJ[j])
    return T


def _split_terms(x):
    terms = []
    r = x
    for _ in range(NTERMS):
        t = r.astype(ml_dtypes.bfloat16)
        terms.append(t)
        r = r - t.astype(np.float32)
    return terms


def _interleave(t0, t1):
    """[R, ...] x2 -> [2R, ...] with rows (2r, 2r+1) = (t0[r], t1[r])."""
    out = np.empty((t0.shape[0] * 2,) + t0.shape[1:], dtype=t0.dtype)
    out[0::2] = t0
    out[1::2] = t1
    return out


def _prep_core(Cb, Sb):
    """c2 [128, NB*3, W]: c2[p, 3b+k, n] = term_{p%2}(C)[128b-32+64k+p//2, n];
    stw2 [128, NB, 12, 128]: stw2[p, ib, a, m] = term_{p%2}(S)[128ib+m,
    64a+p//2]."""
    ct = _split_terms(Cb)
    cint = _interleave(ct[0].astype(np.float32),
                       ct[1].astype(np.float32))  # [1536, W] rows 2v+t
    cpad = np.vstack([np.zeros((64, W), np.float32), cint,
                      np.zeros((192, W), np.float32)])  # row 2(v+32)+t
    c2 = np.zeros((128, NB * 3, W), dtype=np.float32)
    for b in range(NB):
        for k in range(3):
            r0 = 2 * (128 * b + 64 * k)  # = 2*(v0+32) with v0 = 128b-32+64k
            c2[:, 3 * b + k, :] = cpad[r0:r0 + 128, :]
    st = _split_terms(Sb.T)  # [w', u]
    sint = _interleave(st[0].astype(np.float32),
                       st[1].astype(np.float32))  # [1536, u] rows 2w'+t
    stw2 = sint.reshape(12, 128, NB, 128).transpose(1, 2, 0, 3)
    return {"c2": c2.astype(ml_dtypes.bfloat16).reshape(128, -1),
            "stw2": np.ascontiguousarray(stw2).astype(
                ml_dtypes.bfloat16).reshape(128, -1)}


def _consts():
    return {
        "TV2": _toeplitz_v2().astype(ml_dtypes.bfloat16).reshape(128, -1),
        "TH2": _band_h2().astype(ml_dtypes.bfloat16).reshape(128, -1),
        "IDT": np.eye(128, dtype=np.float32),
    }


# ---------------------------------------------------------------- kernel IR
def _build():
    nc = bacc.Bacc()
    C2D = nc.declare_dram_parameter("c2", [128, NB * 3 * W], BF16,
                                    isOutput=False)
    STW2 = nc.declare_dram_parameter("stw2", [128, NB * 12 * 128], BF16,
                                     isOutput=False)
    TV2D = nc.declare_dram_parameter("TV2", [128, NJ * 3 * 128], BF16,
                                     isOutput=False)
    TH2D = nc.declare_dram_parameter("TH2", [128, NJ * 128], BF16,
                                     isOutput=False)
    IDT = nc.declare_dram_parameter("IDT", [128, 128], F32, isOutput=False)
    CONV = nc.declare_dram_parameter("conv", [H, W], F32, isOutput=True)
    MASK = nc.declare_dram_parameter("mask", [H, W], U8, isOutput=True)

    def hspan(a, j):
        # out-col span covered by w'-chunk a (64 cols, both terms)
        return max(0, 64 * a - XJ[j]), min(W, 64 * a + 64 + XJ[j])

    with tile.TileContext(nc) as tc:
        with tc.tile_pool(name="big", bufs=1) as big, \
             tc.tile_pool(name="consts", bufs=1) as cst, \
             tc.tile_pool(name="posg", bufs=1) as posp, \
             tc.tile_pool(name="pool", bufs=2) as poolp, \
             tc.tile_pool(name="atg", bufs=1) as atgp, \
             tc.tile_pool(name="pooled", bufs=1) as pooledp, \
             tc.tile_pool(name="small", bufs=2) as smallp, \
             tc.tile_pool(name="ps", bufs=3, space="PSUM") as ps, \
             tc.tile_pool(name="psT", bufs=2, space="PSUM") as psT:

            c2 = big.tile([128, NB * 3, W], BF16, tag="c2", name="c2")
            stw2 = big.tile([128, NB, 12, 128], BF16, tag="stw2", name="stw2")
            tv2 = cst.tile([128, NJ * 3, 128], BF16, tag="tv2")
            th2 = cst.tile([128, NJ * 128], BF16, tag="th2")
            idt = cst.tile([128, 128], F32, tag="idt")
            # block-priority loads on BOTH hwdge queues: wave ib needs
            # c2[:, 3ib:3ib+3, :] (V) and stw2[:, ib, :, :] (H stationaries).
            # First-wave critical path: TV2 j0-2 slice, then c2 b0 per k-chunk.
            nc.sync.dma_start(out=tv2[:, 0:3, :], in_=TV2D[:, 0:3 * 128])
            nc.scalar.dma_start(out=th2[:], in_=TH2D[:])
            for k in range(3):
                nc.sync.dma_start(out=c2[:, k:k + 1, :],
                                  in_=C2D[:, k * W:(k + 1) * W])
            nc.sync.dma_start(out=tv2[:, 3:21, :], in_=TV2D[:, 3 * 128:])
            nc.scalar.dma_start(out=stw2[:, 0, :, :], in_=STW2[:, 0:12 * 128])
            nc.scalar.dma_start(out=idt[:], in_=IDT[:])
            for b in range(1, NB):
                nc.sync.dma_start(out=c2[:, 3 * b:3 * (b + 1), :],
                                  in_=C2D[:, 3 * b * W:3 * (b + 1) * W])
                nc.scalar.dma_start(
                    out=stw2[:, b, :, :],
                    in_=STW2[:, 12 * 128 * b:12 * 128 * (b + 1)])

            posg = [posp.tile([128, 800], F32, tag=f"posg{ib}", name=f"posg{ib}")
                    for ib in range(NB)]
            atg = atgp.tile([128, NB, 800], F32, tag="atg", name="atg")
            ptv = pooledp.tile([128, NB, W], F32, tag="ptv", name="ptv")
            nc.vector.memset(atg[:, :, 0:16], NEG)
            nc.vector.memset(atg[:, :, 784:800], NEG)

            def last_a(j, h):
                lo_h, hi_h = 384 * h, 384 * (h + 1)
                return max(a for a in range(12)
                           if max(hspan(a, j)[0], lo_h)
                           < min(hspan(a, j)[1], hi_h))

            def emit_wave(ib):
                for (j0, j1) in JG:
                    # 2-bank tiles: [128, 2, 384] padded to [128, 2, 512] so
                    # each half sits bank-aligned; combine reads both at once.
                    ptiles = {j: ps.tile([128, 2, 384], F32, tag="p",
                                         name=f"p{j}",
                                         padded_shape=[128, 2, 512])
                              for j in range(j0, j1)}
                    # ---- V: 3 K-chunks (64 v-rows x 2 terms each) per half
                    for j in range(j0, j1):
                        for k in range(3):
                            for h in range(2):
                                rhs = c2[:, 3 * ib + k, 384 * h:384 * (h + 1)]
                                nc.tensor.matmul(
                                    ptiles[j][:, h, :], tv2[:, 3 * j + k, :],
                                    rhs, start=(k == 0), stop=False,
                                    skip_group_check=True)
                    # ---- H: stationary = stw2 chunk (ib, a); both terms ride
                    spans = {}
                    for a in range(12):
                        for j in range(j0, j1):
                            lo, hi = hspan(a, j)
                            if lo < hi:
                                spans[(a, j)] = (lo, hi)
                    lasts = {}
                    for (a, j) in spans:
                        lasts[j] = a
                    for a in range(12):
                        if not any((a, j) in spans for j in range(j0, j1)):
                            continue
                        lhs = stw2[:, ib, a, :]
                        first = True
                        for j in range(j0, j1):
                            if (a, j) not in spans:
                                continue
                            lo, hi = spans[(a, j)]
                            for h in range(2):
                                l2 = max(lo, 384 * h)
                                h2 = min(hi, 384 * (h + 1))
                                if l2 >= h2:
                                    continue
                                off = 128 * j + (l2 - (64 * a - 32))
                                stop = (lasts[j] == a and h2 == hi)
                                mm = nc.tensor.matmul(
                                    ptiles[j][:, h, l2 - 384 * h:h2 - 384 * h],
                                    lhs, th2[:, off:off + (h2 - l2)],
                                    start=False, stop=stop,
                                    skip_group_check=True)
                                if not first:
                                    mm.ins.ldweights = False
                                first = False
                    # ---- combine into posg, 768 wide via the 2-bank AP
                    dst = posg[ib][:, 16:784]
                    for j in range(j0, j1):
                        if j == 0:
                            nc.scalar.activation(dst, ptiles[0][:, :, :],
                                                 AF.Copy,
                                                 scale=float(SCALES[0]))
                        else:
                            nc.vector.scalar_tensor_tensor(
                                dst, ptiles[j][:, :, :], float(SCALES[j]),
                                dst, ALU.mult, ALU.max)

            def emit_mwh(ib):
                nc.vector.memset(posg[ib][:, 0:16], NEG)
                nc.vector.memset(posg[ib][:, 784:800], NEG)
                m2 = poolp.tile([128, 800], F32, tag="m2", name="m2")
                m4 = poolp.tile([128, 800], F32, tag="m4", name="m4")
                m8 = poolp.tile([128, 800], F32, tag="m8", name="m8")
                a = poolp.tile([128, W], F32, tag="a", name="a")
                g = posg[ib]
                nc.vector.tensor_tensor(m2[:, 0:799], g[:, 0:799], g[:, 1:800],
                                        ALU.max)
                nc.vector.tensor_tensor(m4[:, 0:797], m2[:, 0:797],
                                        m2[:, 2:799], ALU.max)
                nc.vector.tensor_tensor(m8[:, 0:793], m4[:, 0:793],
                                        m4[:, 4:797], ALU.max)
                nc.vector.tensor_tensor(a[:], m8[:, 11:779], m4[:, 18:786],
                                        ALU.max)
                return a

            def emit_at_half(ib, a, half):
                pt = psT.tile([128, 384], F32, tag="pt", name="pt")
                for k in range(3):
                    c = 3 * half + k
                    nc.tensor.transpose(pt[:, 128 * k:128 * (k + 1)],
                                        a[:, 128 * c:128 * (c + 1)], idt[:])
                nc.scalar.activation(
                    atg[:, 3 * half:3 * half + 3,
                        16 + 128 * ib:16 + 128 * (ib + 1)],
                    pt[:], AF.Copy)

            def emit_at(ib, a):
                for half in range(2):
                    emit_at_half(ib, a, half)

            def emit_mwh_half(g, m2, m4, m8, a, half):
                # same 11-window chain as emit_mwh, restricted to one
                # 384-col output half so at() transposes can interleave
                r0 = 11 + 384 * half
                nc.vector.tensor_tensor(m2[:, r0:r0 + 393], g[:, r0:r0 + 393],
                                        g[:, r0 + 1:r0 + 394], ALU.max)
                nc.vector.tensor_tensor(m4[:, r0:r0 + 391], m2[:, r0:r0 + 391],
                                        m2[:, r0 + 2:r0 + 393], ALU.max)
                nc.vector.tensor_tensor(m8[:, r0:r0 + 384], m4[:, r0:r0 + 384],
                                        m4[:, r0 + 4:r0 + 388], ALU.max)
                nc.vector.tensor_tensor(a[:, 384 * half:384 * half + 384],
                                        m8[:, r0:r0 + 384],
                                        m4[:, r0 + 7:r0 + 391], ALU.max)

            def emit_mwv(vib, c0=0, c1=NB):
                # vertical window max; [c0, c1) selects the column chunks so
                # the drain tail can interleave with ptt transposes per half
                av = 16 + 128 * vib
                n = 128
                m2 = poolp.tile([128, NB, 272], F32, tag="m2v", name="m2v")
                m4 = poolp.tile([128, NB, 272], F32, tag="m4v", name="m4v")
                m8 = poolp.tile([128, NB, 272], F32, tag="m8v", name="m8v")
                nc.vector.tensor_tensor(m2[:, c0:c1, 0:n + 16],
                                        atg[:, c0:c1, av - 8:av + n + 8],
                                        atg[:, c0:c1, av - 7:av + n + 9],
                                        ALU.max)
                nc.vector.tensor_tensor(m4[:, c0:c1, 0:n + 14],
                                        m2[:, c0:c1, 0:n + 14],
                                        m2[:, c0:c1, 2:n + 16], ALU.max)
                nc.vector.tensor_tensor(m8[:, c0:c1, 2:n + 10],
                                        m4[:, c0:c1, 2:n + 10],
                                        m4[:, c0:c1, 6:n + 14], ALU.max)
                nc.vector.tensor_tensor(ptv[:, c0:c1, 128 * vib:128 * vib + n],
                                        m8[:, c0:c1, 3:n + 3],
                                        m4[:, c0:c1, 10:n + 10], ALU.max)

            def emit_ptt_half(vib, half, mk):
                # fused mask per half, reading the transposed pooled tile
                # straight from PSUM (one PSUM operand is allowed):
                #   mask = (pooled max nextafter(0.5)) <= posg
                # pooled >= posg always, so <= means equality (local max)
                # AND posg > 0.5 via the raised scalar.
                pt = psT.tile([128, 384], F32, tag="pt", name="pt")
                for k in range(3):
                    c = 3 * half + k
                    nc.tensor.transpose(
                        pt[:, 128 * k:128 * (k + 1)],
                        ptv[:, c, 128 * vib:128 * (vib + 1)], idt[:])
                nc.vector.scalar_tensor_tensor(
                    mk[:, 384 * half:384 * (half + 1)], pt[:], C0,
                    posg[vib][:, 16 + 384 * half:16 + 384 * (half + 1)],
                    ALU.max, ALU.is_le)

            def emit_ptt(vib):
                mk = smallp.tile([128, W], U8, tag="mk", name="mk")
                emit_ptt_half(vib, 0, mk)
                emit_ptt_half(vib, 1, mk)
                nc.scalar.dma_start(out=MASK[128 * vib:128 * (vib + 1), :],
                                  in_=mk[:])

            alist = {}
            for ib in range(NB):
                emit_wave(ib)
                nc.sync.dma_start(out=CONV[128 * ib:128 * (ib + 1), :],
                                  in_=posg[ib][:, 16:784])
                # pooling emitted one block behind the conv waves: combine
                # STTs (which free PSUM tiles for the PE) then outrank the
                # pooling TTs in the scheduler's program-order tiebreak,
                # closing the ~2us-per-block PE stalls seen in the trace
                if ib >= 1:
                    alist[ib - 1] = emit_mwh(ib - 1)
                    emit_at(ib - 1, alist[ib - 1])
                if ib >= 2:
                    emit_mwv(ib - 2)
                    emit_ptt(ib - 2)
            # drain: last block's mwh/at interleaved halves
            ib = NB - 1
            nc.vector.memset(posg[ib][:, 0:16], NEG)
            nc.vector.memset(posg[ib][:, 784:800], NEG)
            m2 = poolp.tile([128, 800], F32, tag="m2", name="m2")
            m4 = poolp.tile([128, 800], F32, tag="m4", name="m4")
            m8 = poolp.tile([128, 800], F32, tag="m8", name="m8")
            a = poolp.tile([128, W], F32, tag="a", name="a")
            g = posg[ib]
            emit_mwh_half(g, m2, m4, m8, a, 0)
            emit_at_half(ib, a, 0)
            emit_mwh_half(g, m2, m4, m8, a, 1)
            emit_at_half(ib, a, 1)
            # drain tail: split mwv by chunk-halves and interleave the ptt
            # transposes+mask so PE/DVE overlap instead of serializing
            for vib in (NB - 2, NB - 1):
                mk = smallp.tile([128, W], U8, tag="mk", name="mk")
                emit_mwv(vib, 0, 3)
                emit_ptt_half(vib, 0, mk)
                emit_mwv(vib, 3, 6)
                emit_ptt_half(vib, 1, mk)
                nc.scalar.dma_start(out=MASK[128 * vib:128 * (vib + 1), :],
                                    in_=mk[:])

    nc.compile()
    return nc


# ---------------------------------------------------------------- host glue
def kernel(C, S, kernel_cos, kernel_sin):
    C = np.asarray(C, dtype=np.float32)
    S = np.asarray(S, dtype=np.float32)
    B = C.shape[0]
    if "nc" not in _CACHE:
        _CACHE["nc"] = _build()
    nc = _CACHE["nc"]
    consts = _consts()
    in_maps = []
    for b in range(B):
        m = _prep_core(C[b, 0], S[b, 0])
        m.update(consts)
        in_maps.append(m)
    res = run_bass_kernel_spmd(nc, in_maps, core_ids=list(range(B)))
    conv = np.stack([r["conv"] for r in res.results])[:, None]
    mask = np.stack([r["mask"] for r in res.results])[:, None].astype(bool)
    return conv.astype(np.float32), mask

